# revision 14
# baseline (speedup 1.0000x reference)
"""Trainium2 Bass kernel for nn_MILLoss (min-instance loss over label bags).

Math: raw_loss[i] = logsumexp(logits[i,:]) - logits[i, tgt[i]]  (CE, all valid)
      seg_min[c]  = min_{i: tgt[i]=c} raw_loss[i]
      out         = mean_{c present}(seg_min[c])

Host casts logits to f16 (|x| < ~6.5 for N(0,1) inputs; ~5e-4 rel err on exp,
~1e-3 abs on the loss vs the 2e-2 gate), halving HBM traffic: 32 MiB/core
streams in ~82us at the measured ~410 GB/s/core. Host also gathers the target
logit x_t[i] = logits[i, tgt[i]] (O(B) numpy) - the device computes only the
row logsumexp denominators Z, which is the O(B*C) memory-bound crunch.

Device (per core, B_core = 16384 rows = 128 tiles of 128 rows, 2MB chunks of
8 tiles): Act runs ONE batched exp per 8-tile chunk (amortizes the ~352-cycle
instruction overhead; ~0.89us/tile, ~114us total). DVE reduces each tile to
Z[:, t] via a 2x-mode pairwise tree fold (tensor_tensor adds 1024->512->256
->128) followed by a 1x tensor_scalar accum over the last 128 elements
(~0.87us/tile, ~111us total) - cheaper than a full-width 1x accum (1.13us).
Both engines overlap the ~82us DMA stream.

Host: raw_loss = ln Z - x_t, numpy segment-min keyed on target, mean over
present labels.
"""

import numpy as np

P = 128          # SBUF partitions
C = 1024         # num classes
NCORES = 8
B = 131072
B_CORE = B // NCORES      # 16384
T = B_CORE // P           # 128 tiles of 128 rows per core
CHUNK = 4                 # tiles per DMA transfer (1 MB f16)

_cache = {}


def _build(n_tiles, reps=1, loop=None, chunk=CHUNK):
    """Per-core Bass program (SPMD, same program all cores).

    reps>1 unrolls the body; loop=R wraps it in a device-side For_i
    (idempotent rewrites - used for wall-clock differencing benchmarks).
    """
    import concourse.bacc as bacc
    import concourse.tile as tile
    from concourse import mybir

    f32, f16 = mybir.dt.float32, mybir.dt.float16
    Act = mybir.ActivationFunctionType
    Op = mybir.AluOpType
    NCH = n_tiles // chunk
    # activation groups of 12 tiles (10.5us quanta) + one 8-tile remainder
    groups = []
    t0 = 0
    while t0 < n_tiles:
        g = min(12, n_tiles - t0)
        if n_tiles - t0 == 16:
            g = 8
        groups.append((t0, g))
        t0 += g

    nc = bacc.Bacc(None)
    lg = nc.declare_dram_parameter("logits", [P * n_tiles, C], f16, isOutput=False)
    zout = nc.declare_dram_parameter("zout", [P, n_tiles], f32, isOutput=True)

    # chunk u covers rows [u*chunk*128, (u+1)*chunk*128): contiguous in HBM
    lgv = lg.rearrange("(u b p) c -> u p b c", b=chunk, p=P)

    with tile.TileContext(nc) as tc:
        with (
            tc.tile_pool(name="consts", bufs=1) as consts,
            tc.tile_pool(name="xp", bufs=4) as xp,
            tc.tile_pool(name="ep", bufs=3) as ep,
            tc.tile_pool(name="fp1", bufs=4) as fp1,
            tc.tile_pool(name="fp2", bufs=4) as fp2,
            tc.tile_pool(name="fp3", bufs=4) as fp3,
            tc.tile_pool(name="sp", bufs=4) as sp,
        ):
            z_sb = consts.tile([P, n_tiles], f32)

            def body():
                for t0, g in [grp for _ in range(reps) for grp in groups]:
                    # group of g tiles assembled from g//chunk 1MB DMAs,
                    # exp'd by ONE activation (amortizes the ~352c overhead)
                    xt = xp.tile([P, g, C], f16, tag="xt")
                    for j in range(g // chunk):
                        nc.sync.dma_start(
                            xt[:, j * chunk : (j + 1) * chunk, :],
                            lgv[t0 // chunk + j])
                    e = ep.tile([P, g, C], f16, tag="e")
                    nc.scalar.activation(e[:, :, :], xt[:, :, :], Act.Exp)
                    for b in range(g):
                        t = t0 + b
                        f1 = fp1.tile([P, 512], f16)
                        nc.vector.tensor_tensor(
                            f1[:, :], e[:, b, 0:512], e[:, b, 512:1024], Op.add)
                        f2 = fp2.tile([P, 256], f16)
                        nc.vector.tensor_tensor(
                            f2[:, :], f1[:, 0:256], f1[:, 256:512], Op.add)
                        f3 = fp3.tile([P, 128], f16)
                        nc.vector.tensor_tensor(
                            f3[:, :], f2[:, 0:128], f2[:, 128:256], Op.add)
                        s2 = sp.tile([P, 128], f16)
                        nc.vector.tensor_scalar(
                            s2[:, :], f3[:, :], 1.0, 0.0, Op.mult, Op.add,
                            accum_out=z_sb[:, t : t + 1],
                        )

            if loop is not None:
                with tc.For_i(0, loop, 1):
                    body()
            else:
                body()

            nc.sync.dma_start(zout[:, :], z_sb[:, :])
    nc.compile()
    return nc


def _get_nc(n_tiles):
    if n_tiles not in _cache:
        _cache[n_tiles] = _build(n_tiles)
    return _cache[n_tiles]


def _make_in_maps(logits, target, n_tiles, n_cores):
    logits = np.asarray(logits, dtype=np.float32).astype(np.float16)
    b_core = P * n_tiles
    in_maps = []
    for k in range(n_cores):
        sh_l = np.ascontiguousarray(logits[k * b_core : (k + 1) * b_core])
        in_maps.append({"logits": sh_l})
    return in_maps


def _combine(z_list, logits, target, n_tiles):
    """z_list: per-core [128, T] f32 rowsum-exp; local row = t*128 + p."""
    z = np.stack(z_list)                                # [ncores, P, T]
    lnz = np.log(z.astype(np.float64)).transpose(0, 2, 1).reshape(-1)  # [B]
    tgt = np.asarray(target).astype(np.int64)
    # target logit, from the same f16-cast values the device streamed
    logits16 = np.asarray(logits, dtype=np.float32).astype(np.float16)
    x_t = np.take_along_axis(logits16, tgt[:, None], axis=1)[:, 0]
    raw = lnz - x_t.astype(np.float64)
    seg = np.full((C,), np.inf)
    np.minimum.at(seg, tgt, raw)
    present = seg != np.inf
    n = int(present.sum())
    if n == 0:
        return np.float32(0.0)
    return np.float32(seg[present].sum() / n)


def kernel(logits, target):
    from concourse.bass_utils import run_bass_kernel_spmd

    nc = _get_nc(T)
    in_maps = _make_in_maps(logits, target, T, NCORES)
    res = run_bass_kernel_spmd(nc, in_maps, core_ids=list(range(NCORES)))
    return _combine([r["zout"] for r in res.results], logits, target, T)
